# revision 13
# baseline (speedup 1.0000x reference)
"""Multi-head self-attention (B=2, S=1024, D=1024, H=16) on 8 TRN2 NeuronCores.

Sharding: tensor-parallel over heads (2 heads/core, both batch elements),
Megatron-style. Per core:
  1. QKV^T projection (bf16 matmuls): Q^T, K^T (head dims on partitions) and
     V^T, which is PE-transposed to V-natural layout with an appended ones
     column (yields softmax denominators for free during attn@V).
  2. scores^T = K_h Q_h^T per (head, batch) with k-tokens on partitions;
     exp on ACT (no max subtraction -- logits are O(1) by construction);
     attn@V accumulated over k-tiles -> unnormalized attnout^T + denom row.
     Batch-1 QKV matmuls are interleaved into batch-0 attention as PE filler
     so the ACT-paced stretch keeps the TensorEngine busy.
  3. Normalize via fp32r reciprocal + PE broadcast. Two 8-rank AllToAlls
     (one per local head, issued as soon as that head finishes for both
     batches) turn head-sharding into token-sharding while the other head's
     attention still computes. The output projection runs as two K=64
     accumulation passes (one per AllToAll payload) with the full W_out.
Host assembles the 8 [1024 e, 256 s] shards into [2, 1024, 1024].
"""
import sys

sys.path.insert(0, "/opt/trn_rl_repo")

import numpy as np
import ml_dtypes

B, S, D, H = 2, 1024, 1024, 16
DH = D // H
N_CORES = 8
SCALE = 1.0 / float(np.sqrt(DH))
NT = S // 128   # token tiles per batch
NDT = D // 128  # d tiles

BF16 = ml_dtypes.bfloat16

_nc_cache = {}


def _build_nc(iters=1):
    if iters in _nc_cache:
        return _nc_cache[iters]

    from contextlib import ExitStack

    import concourse.bacc as bacc
    import concourse.mybir as mybir
    import concourse.tile as tile

    F32 = mybir.dt.float32
    F32R = mybir.dt.float32r
    BF = mybir.dt.bfloat16
    EXP = mybir.ActivationFunctionType.Exp

    nc = bacc.Bacc(None, target_bir_lowering=False)

    xT = nc.dram_tensor("xT", [B, D, S], BF, kind="ExternalInput")
    wqkvT = nc.dram_tensor("wqkvT", [D, 384], BF, kind="ExternalInput")
    woT = nc.dram_tensor("woT", [D, D], BF, kind="ExternalInput")
    onesB = nc.dram_tensor("onesB", [128, 2], BF, kind="ExternalInput")
    # fp32 ones; bitcast to f32r for the PE normalization broadcast. Also
    # doubles as the bf16 transpose identity via columns 2:66? No -- identity
    # needs its own layout, keep a separate input.
    ident = nc.dram_tensor("ident", [128, 128], BF, kind="ExternalInput")
    onesF = nc.dram_tensor("onesF", [128, 66], F32, kind="ExternalInput")
    outT = nc.dram_tensor("outT", [D, S * B // N_CORES], F32, kind="ExternalOutput")

    a2a_in = [[nc.dram_tensor(f"a2a_in{i}_{hl}", [N_CORES * 64, 256], BF)
               for hl in range(2)] for i in range(iters)]
    a2a_out = [[nc.dram_tensor(f"a2a_out{i}_{hl}", [N_CORES * 64, 256], BF)
                for hl in range(2)] for i in range(iters)]

    with tile.TileContext(nc) as tc, nc.allow_low_precision(reason="bf16 attention"):
      for it in range(iters):
        with ExitStack() as stack:
            const_pool = stack.enter_context(tc.tile_pool(name="const", bufs=1))
            wk_pool = stack.enter_context(tc.tile_pool(name="wk", bufs=1))
            wo_pool = stack.enter_context(tc.tile_pool(name="wo", bufs=1))
            qk_pool = stack.enter_context(tc.tile_pool(name="qk", bufs=4))
            v65_pool = stack.enter_context(tc.tile_pool(name="v65", bufs=B * NT))
            xt_pool = stack.enter_context(tc.tile_pool(name="xt", bufs=B))
            vt_pool = stack.enter_context(tc.tile_pool(name="vt", bufs=B))
            exp_pool = stack.enter_context(tc.tile_pool(name="expt", bufs=12))
            att_pool = stack.enter_context(tc.tile_pool(name="att", bufs=2))
            rcp_pool = stack.enter_context(tc.tile_pool(name="rcp", bufs=2))
            bcs_pool = stack.enter_context(tc.tile_pool(name="bcs", bufs=2))
            af_pool = stack.enter_context(tc.tile_pool(name="af", bufs=2))
            osb_pool = stack.enter_context(tc.tile_pool(name="osb", bufs=NDT))
            # PSUM budget (8 banks): QKV accumulators 1x2, attn@V accumulators
            # 1x2, scores/transposes/broadcast 2x2.  The out-projection reuses
            # the QKV and attn@V slots (alternating) once attention is done.
            pq_pool = stack.enter_context(
                tc.tile_pool(name="pq", bufs=1, space="PSUM"))
            pa_pool = stack.enter_context(
                tc.tile_pool(name="pa", bufs=1, space="PSUM"))
            sc_pool = stack.enter_context(
                tc.tile_pool(name="sc", bufs=2, space="PSUM"))

            idt = const_pool.tile([128, 128], BF, tag="ident")
            nc.sync.dma_start(idt[:], ident[:])
            ones_b = const_pool.tile([128, 2], BF, tag="onesb")
            nc.sync.dma_start(ones_b[:], onesB[:])
            ones_f = const_pool.tile([128, 66], F32R, tag="onesf")
            nc.sync.dma_start(ones_f[:], onesF[:].bitcast(F32R))

            wkb = wk_pool.tile([128, NDT * 384], BF, tag="wk", name="wkb")
            for chunk in range(2):
                d0 = chunk * (NDT // 2)
                nc.sync.dma_start(
                    wkb[:, d0 * 384:(d0 + NDT // 2) * 384].rearrange(
                        "p (dt c) -> p dt c", c=384),
                    wqkvT[d0 * 128:(d0 + NDT // 2) * 128, :].rearrange(
                        "(dt p) c -> p dt c", p=128),
                )

            xt = []
            for b in range(B):
                t = xt_pool.tile([128, NDT * S], BF, tag="xt", name=f"xtb{b}")
                bounds = [0, 1, 2, 3, 4, 5, 6, 8] if b == 0 else [0, 4, 8]
                for c0, c1 in zip(bounds[:-1], bounds[1:]):
                    nc.sync.dma_start(
                        t[:, c0 * S:c1 * S].rearrange(
                            "p (dt s) -> p dt s", s=S),
                        xT[b, c0 * 128:c1 * 128, :].rearrange(
                            "(dt p) s -> p dt s", p=128),
                    )
                xt.append(t)

            wob = wo_pool.tile([128, NDT * D], BF, tag="wo", name="wob")
            for chunk in range(2):
                d0 = chunk * (NDT // 2)
                nc.sync.dma_start(
                    wob[:, d0 * D:(d0 + NDT // 2) * D].rearrange(
                        "p (dt c) -> p dt c", c=D),
                    woT[d0 * 128:(d0 + NDT // 2) * 128, :].rearrange(
                        "(dt p) c -> p dt c", p=128),
                )

            qt = [qk_pool.tile([128, S], BF, tag="qk", name=f"qt{b}")
                  for b in range(B)]
            kt = [qk_pool.tile([128, S], BF, tag="qk", name=f"kt{b}")
                  for b in range(B)]
            v65 = [[v65_pool.tile([128, 2 * (DH + 1)], BF, tag="v65",
                                  name=f"v65_{b}_{tb}")
                    for tb in range(NT)] for b in range(B)]
            att = [att_pool.tile([64, B * S], BF, tag="att", name=f"att{hl}")
                   for hl in range(2)]

            # ---------------- emission helpers ----------------
            def qkv_batch(b, fillers=None):
                """QKV^T for batch b. If fillers is not None, emit closures
                that each do one accumulation-group's worth of work so the
                caller can interleave them into attention; else emit now."""
                ops = []
                for part, dst in ((0, qt[b]), (1, kt[b]), (2, None)):
                    def group(part=part, dst=dst, b=b):
                        ps = pq_pool.tile([128, S], F32, tag="pq", name=f"qkv{b}_{part}")
                        for dt in range(NDT):
                            for nb in range(2):
                                nc.tensor.matmul(
                                    ps[:, nb * 512:(nb + 1) * 512],
                                    wkb[:, dt * 384 + part * 128:
                                        dt * 384 + (part + 1) * 128],
                                    xt[b][:, dt * S + nb * 512:
                                          dt * S + (nb + 1) * 512],
                                    start=(dt == 0),
                                    stop=(dt == NDT - 1),
                                )
                        if dst is not None:
                            nc.vector.tensor_copy(dst[:], ps[:])
                            return None
                        v = vt_pool.tile([128, S], BF, tag="vt", name=f"vt{b}")
                        nc.vector.tensor_copy(v[:], ps[:])
                        return v

                    ops.append(group)

                return ops

            vt_tiles = {}

            def emit_qkv_group(fn, b):
                v = fn()
                if v is not None:
                    vt_tiles[b] = v

            def emit_transpose(b, tb):
                pst = sc_pool.tile([128, 128], BF, tag="sc", name=f"pst{b}_{tb}")
                nc.tensor.transpose(
                    pst[:], vt_tiles[b][:, tb * 128:(tb + 1) * 128], idt[:]
                )
                dst = v65[b][tb]
                for hl in range(2):
                    nc.vector.tensor_copy(
                        dst[:, hl * 65:hl * 65 + 64],
                        pst[:, hl * 64:(hl + 1) * 64],
                    )
                    nc.vector.tensor_copy(
                        dst[:, hl * 65 + 64:hl * 65 + 65],
                        ones_b[:, hl:hl + 1],
                    )

            def scores_phase(b, hl, fillers):
                """scores + exp for all k-blocks of one (batch, head) instance;
                pops one filler closure per k-block to keep PE fed while ACT
                paces the exps."""
                p0 = hl * 64
                ets = []
                for kb in range(NT):
                    psc = sc_pool.tile([128, S], F32, tag="sc",
                                       name=f"psc{b}_{hl}_{kb}")
                    for nb in range(2):
                        nc.tensor.matmul(
                            psc[:, nb * 512:(nb + 1) * 512],
                            kt[b][p0:p0 + 64, kb * 128:(kb + 1) * 128],
                            qt[b][p0:p0 + 64, nb * 512:(nb + 1) * 512],
                            start=True,
                            stop=True,
                        )
                    et = exp_pool.tile([128, S], BF, tag="expt",
                                       name=f"et{b}_{hl}_{kb}")
                    nc.scalar.activation(et[:], psc[:], EXP)
                    ets.append(et)
                    if fillers:
                        fillers.pop(0)()
                return ets

            def attnv_phase(b, hl, ets, fillers):
                """attn@V accumulation + normalization for one instance."""
                pav = pa_pool.tile([65, S], F32, tag="pa", name=f"pav{b}_{hl}")
                for kb in range(NT):
                    for nb in range(2):
                        nc.tensor.matmul(
                            pav[:, nb * 512:(nb + 1) * 512],
                            v65[b][kb][:, hl * 65:(hl + 1) * 65],
                            ets[kb][:, nb * 512:(nb + 1) * 512],
                            start=(kb == 0),
                            stop=(kb == NT - 1),
                        )
                    if fillers:
                        fillers.pop(0)()
                # normalize: f32r reciprocal + PE broadcast + multiply
                rc = rcp_pool.tile([65, S], F32R, tag="rcp", name=f"rc{b}_{hl}")
                nc.vector.reciprocal(rc[64:65, :], pav[64:65, :].bitcast(F32R))
                pbc = sc_pool.tile([64, S], F32, tag="sc", name=f"pbc{b}_{hl}")
                for nb in range(2):
                    nc.tensor.matmul(
                        pbc[:, nb * 512:(nb + 1) * 512],
                        ones_f[64:65, 2:66],
                        rc[64:65, nb * 512:(nb + 1) * 512],
                        start=True,
                        stop=True,
                    )
                bc = bcs_pool.tile([64, S], F32, tag="bcs", name=f"bc{b}_{hl}")
                nc.vector.tensor_copy(bc[:], pbc[:])
                nc.vector.tensor_mul(
                    att[hl][:, b * S:(b + 1) * S], pav[:64, :], bc[:]
                )

            def a2a(hl):
                nc.sync.dma_start(
                    a2a_in[it][hl][:].rearrange("(j p) s -> p j s", p=64),
                    att[hl][:].rearrange("p (j s) -> p j s", s=256),
                )
                nc.gpsimd.collective_compute(
                    "AllToAll",
                    mybir.AluOpType.bypass,
                    replica_groups=[list(range(N_CORES))],
                    ins=[a2a_in[it][hl][:]],
                    outs=[a2a_out[it][hl][:]],
                )
                af = af_pool.tile([128, N_CORES * 256], BF, tag="af",
                                  name=f"af{hl}")
                nc.sync.dma_start(
                    af[hl * 64:(hl + 1) * 64, :].rearrange(
                        "p (j s) -> p j s", s=256),
                    a2a_out[it][hl][:].rearrange("(j p) s -> p j s", p=64),
                )
                return af

            # ---------------- emission ----------------
            # batch-0 Q and K eagerly; everything else slots in as PE filler
            # between ACT-paced score/exp blocks.
            g0 = qkv_batch(0)
            g1 = qkv_batch(1)
            emit_qkv_group(g0[0], 0)   # Q(b0)
            emit_qkv_group(g0[1], 0)   # K(b0)

            fillers = [lambda: emit_qkv_group(g0[2], 0)]          # V(b0)
            fillers += [lambda tb=tb: emit_transpose(0, tb) for tb in range(NT)]
            ets00 = scores_phase(0, 0, fillers)        # pops V(b0)+T(b0)x7
            # Q/K of batch 1 right after batch-0 scores: PE fills the stretch
            # where ACT still works through the batch-0 exps.
            emit_qkv_group(g1[0], 1)
            emit_qkv_group(g1[1], 1)
            fillers += [lambda: emit_qkv_group(g1[2], 1)]         # V(b1)
            fillers += [lambda tb=tb: emit_transpose(1, tb) for tb in range(NT)]
            attnv_phase(0, 0, ets00, fillers)
            ets10 = scores_phase(1, 0, fillers)
            attnv_phase(1, 0, ets10, fillers)
            while fillers:
                fillers.pop(0)()

            af0 = a2a(0)   # hl=0 payload ships while hl=1 computes

            ets01 = scores_phase(0, 1, None)
            attnv_phase(0, 1, ets01, None)
            ets11 = scores_phase(1, 1, None)
            attnv_phase(1, 1, ets11, None)

            af1 = a2a(1)

            # ---- output projection: two K=64 passes over the A2A payloads
            opart = [osb_pool.tile([128, 256], F32, tag="opart", name=f"op{eb}",
                                   bufs=NDT)
                     for eb in range(NDT)]
            for eb in range(NDT):
                pool, tag = (pq_pool, "pq") if eb % 2 == 0 else (pa_pool, "pa")
                po = pool.tile([128, 256], F32, tag=tag, name=f"po1_{eb}")
                for j in range(N_CORES):
                    nc.tensor.matmul(
                        po[:],
                        wob[0:64, j * D + eb * 128:j * D + (eb + 1) * 128],
                        af0[0:64, j * 256:(j + 1) * 256],
                        start=(j == 0),
                        stop=(j == N_CORES - 1),
                    )
                nc.vector.tensor_copy(opart[eb][:], po[:])
            for eb in range(NDT):
                pool, tag = (pq_pool, "pq") if eb % 2 == 0 else (pa_pool, "pa")
                po = pool.tile([128, 256], F32, tag=tag, name=f"po2_{eb}")
                for j in range(N_CORES):
                    nc.tensor.matmul(
                        po[:],
                        wob[64:128, j * D + eb * 128:j * D + (eb + 1) * 128],
                        af1[64:128, j * 256:(j + 1) * 256],
                        start=(j == 0),
                        stop=(j == N_CORES - 1),
                    )
                ot = osb_pool.tile([128, 256], F32, tag="osb", name=f"ot{eb}")
                nc.vector.tensor_add(ot[:], po[:], opart[eb][:])
                nc.sync.dma_start(outT[eb * 128:(eb + 1) * 128, :], ot[:])

    nc.finalize()
    _nc_cache[iters] = nc
    return nc


def prep_inputs(x, W_qkv, W_out):
    """Build per-core input maps (numpy only)."""
    x = np.asarray(x, dtype=np.float32)
    W_qkv = np.asarray(W_qkv, dtype=np.float32)
    W_out = np.asarray(W_out, dtype=np.float32)

    xT = np.ascontiguousarray(x.transpose(0, 2, 1)).astype(BF16)   # [B, D, S]
    woT = np.ascontiguousarray(W_out.T).astype(BF16)               # [D, D]
    ident = np.eye(128, dtype=BF16)
    onesB = np.ones((128, 2), dtype=BF16)
    onesF = np.ones((128, 66), dtype=np.float32)

    Wr = W_qkv.reshape(3, H, DH, D)
    in_maps = []
    for c in range(N_CORES):
        hs = slice(2 * c, 2 * c + 2)
        wq = Wr[0, hs].reshape(128, D) * SCALE
        wkk = Wr[1, hs].reshape(128, D)
        wv = Wr[2, hs].reshape(128, D)
        wqkvT = np.ascontiguousarray(np.concatenate([wq, wkk, wv], 0).T).astype(BF16)
        in_maps.append({
            "xT": xT,
            "wqkvT": wqkvT,
            "woT": woT,
            "ident": ident,
            "onesB": onesB,
            "onesF": onesF,
        })
    return in_maps


def assemble(results):
    out = np.empty((B, S, D), dtype=np.float32)
    for c in range(N_CORES):
        b, r = c // 4, c % 4
        out[b, r * 256:(r + 1) * 256, :] = results[c]["outT"].T
    return out


def kernel(x, W_qkv, W_out):
    from concourse.bass_utils import run_bass_kernel_spmd

    nc = _build_nc()
    in_maps = prep_inputs(x, W_qkv, W_out)
    res = run_bass_kernel_spmd(nc, in_maps, list(range(N_CORES)))
    return assemble(res.results)


# revision 14
# speedup vs baseline: 1.1057x; 1.1057x over previous
"""Multi-head self-attention (B=2, S=1024, D=1024, H=16) on 8 TRN2 NeuronCores.

Sharding: tensor-parallel over heads (2 heads/core, both batch elements),
Megatron-style. Per core:
  1. QKV^T projection (bf16 matmuls): Q^T, K^T (head dims on partitions) and
     V^T, which is PE-transposed to V-natural layout with an appended ones
     column (yields softmax denominators for free during attn@V).
  2. scores^T = K_h Q_h^T per (head, batch) with k-tokens on partitions;
     exp on ACT (no max subtraction -- logits are O(1) by construction);
     attn@V accumulated over k-tiles -> unnormalized attnout^T + denom row.
     Batch-1 QKV matmuls are interleaved into batch-0 attention as PE filler
     so the ACT-paced stretch keeps the TensorEngine busy.
  3. Normalize via fp32r reciprocal + PE broadcast. Two 8-rank AllToAlls
     (one per local head, issued as soon as that head finishes for both
     batches) turn head-sharding into token-sharding while the other head's
     attention still computes. The output projection runs as two K=64
     accumulation passes (one per AllToAll payload) with the full W_out.
Host assembles the 8 [1024 e, 256 s] shards into [2, 1024, 1024].
"""
import sys

sys.path.insert(0, "/opt/trn_rl_repo")

import numpy as np
import ml_dtypes

B, S, D, H = 2, 1024, 1024, 16
DH = D // H
N_CORES = 8
SCALE = 1.0 / float(np.sqrt(DH))
NT = S // 128   # token tiles per batch
NDT = D // 128  # d tiles

BF16 = ml_dtypes.bfloat16

_nc_cache = {}
SPLIT_A2A = True


def _build_nc(iters=1):
    key = (iters, SPLIT_A2A)
    if key in _nc_cache:
        return _nc_cache[key]

    from contextlib import ExitStack

    import concourse.bacc as bacc
    import concourse.mybir as mybir
    import concourse.tile as tile

    F32 = mybir.dt.float32
    F32R = mybir.dt.float32r
    BF = mybir.dt.bfloat16
    EXP = mybir.ActivationFunctionType.Exp

    nc = bacc.Bacc(None, target_bir_lowering=False)

    xT = nc.dram_tensor("xT", [B, D, S], BF, kind="ExternalInput")
    wqkvT = nc.dram_tensor("wqkvT", [D, 384], BF, kind="ExternalInput")
    woT = nc.dram_tensor("woT", [D, D], BF, kind="ExternalInput")
    onesB = nc.dram_tensor("onesB", [128, 2], BF, kind="ExternalInput")
    # fp32 ones; bitcast to f32r for the PE normalization broadcast. Also
    # doubles as the bf16 transpose identity via columns 2:66? No -- identity
    # needs its own layout, keep a separate input.
    ident = nc.dram_tensor("ident", [128, 128], BF, kind="ExternalInput")
    onesF = nc.dram_tensor("onesF", [128, 66], F32, kind="ExternalInput")
    outT = nc.dram_tensor("outT", [D, S * B // N_CORES], F32, kind="ExternalOutput")

    if SPLIT_A2A:
        a2a_in = [[nc.dram_tensor(f"a2a_in{i}_{hl}", [N_CORES * 64, 256], BF)
                   for hl in range(2)] for i in range(iters)]
        a2a_out = [[nc.dram_tensor(f"a2a_out{i}_{hl}", [N_CORES * 64, 256], BF)
                    for hl in range(2)] for i in range(iters)]
    else:
        a2a_in = [nc.dram_tensor(f"a2a_in{i}", [N_CORES * 128, 256], BF)
                  for i in range(iters)]
        a2a_out = [nc.dram_tensor(f"a2a_out{i}", [N_CORES * 128, 256], BF)
                   for i in range(iters)]

    with tile.TileContext(nc) as tc, nc.allow_low_precision(reason="bf16 attention"):
      for it in range(iters):
        with ExitStack() as stack:
            const_pool = stack.enter_context(tc.tile_pool(name="const", bufs=1))
            wk_pool = stack.enter_context(tc.tile_pool(name="wk", bufs=1))
            wo_pool = stack.enter_context(tc.tile_pool(name="wo", bufs=1))
            qk_pool = stack.enter_context(tc.tile_pool(name="qk", bufs=4))
            v65_pool = stack.enter_context(tc.tile_pool(name="v65", bufs=B * NT))
            xt_pool = stack.enter_context(tc.tile_pool(name="xt", bufs=B))
            vt_pool = stack.enter_context(tc.tile_pool(name="vt", bufs=B))
            exp_pool = stack.enter_context(tc.tile_pool(name="expt", bufs=12))
            att_pool = stack.enter_context(tc.tile_pool(name="att", bufs=2))
            rcp_pool = stack.enter_context(tc.tile_pool(name="rcp", bufs=2))
            bcs_pool = stack.enter_context(tc.tile_pool(name="bcs", bufs=2))
            af_pool = stack.enter_context(tc.tile_pool(name="af", bufs=2))
            osb_pool = stack.enter_context(tc.tile_pool(name="osb", bufs=NDT))
            # PSUM budget (8 banks): QKV accumulators 1x2, attn@V accumulators
            # 1x2, scores/transposes/broadcast 2x2.  The out-projection reuses
            # the QKV and attn@V slots (alternating) once attention is done.
            pq_pool = stack.enter_context(
                tc.tile_pool(name="pq", bufs=1, space="PSUM"))
            pa_pool = stack.enter_context(
                tc.tile_pool(name="pa", bufs=1, space="PSUM"))
            sc_pool = stack.enter_context(
                tc.tile_pool(name="sc", bufs=2, space="PSUM"))

            idt = const_pool.tile([128, 128], BF, tag="ident")
            nc.sync.dma_start(idt[:], ident[:])
            ones_b = const_pool.tile([128, 2], BF, tag="onesb")
            nc.sync.dma_start(ones_b[:], onesB[:])
            ones_f = const_pool.tile([128, 66], F32R, tag="onesf")
            nc.sync.dma_start(ones_f[:], onesF[:].bitcast(F32R))

            wkb = wk_pool.tile([128, NDT * 384], BF, tag="wk", name="wkb")
            for chunk in range(2):
                d0 = chunk * (NDT // 2)
                nc.sync.dma_start(
                    wkb[:, d0 * 384:(d0 + NDT // 2) * 384].rearrange(
                        "p (dt c) -> p dt c", c=384),
                    wqkvT[d0 * 128:(d0 + NDT // 2) * 128, :].rearrange(
                        "(dt p) c -> p dt c", p=128),
                )

            xt = []
            for b in range(B):
                t = xt_pool.tile([128, NDT * S], BF, tag="xt", name=f"xtb{b}")
                bounds = [0, 1, 2, 3, 4, 5, 6, 8] if b == 0 else [0, 4, 8]
                for c0, c1 in zip(bounds[:-1], bounds[1:]):
                    nc.sync.dma_start(
                        t[:, c0 * S:c1 * S].rearrange(
                            "p (dt s) -> p dt s", s=S),
                        xT[b, c0 * 128:c1 * 128, :].rearrange(
                            "(dt p) s -> p dt s", p=128),
                    )
                xt.append(t)

            wob = wo_pool.tile([128, NDT * D], BF, tag="wo", name="wob")
            for chunk in range(2):
                d0 = chunk * (NDT // 2)
                nc.sync.dma_start(
                    wob[:, d0 * D:(d0 + NDT // 2) * D].rearrange(
                        "p (dt c) -> p dt c", c=D),
                    woT[d0 * 128:(d0 + NDT // 2) * 128, :].rearrange(
                        "(dt p) c -> p dt c", p=128),
                )

            qt = [qk_pool.tile([128, S], BF, tag="qk", name=f"qt{b}")
                  for b in range(B)]
            kt = [qk_pool.tile([128, S], BF, tag="qk", name=f"kt{b}")
                  for b in range(B)]
            v65 = [[v65_pool.tile([128, 2 * (DH + 1)], BF, tag="v65",
                                  name=f"v65_{b}_{tb}")
                    for tb in range(NT)] for b in range(B)]
            att = [att_pool.tile([64, B * S], BF, tag="att", name=f"att{hl}")
                   for hl in range(2)]

            # ---------------- emission helpers ----------------
            def qkv_batch(b, fillers=None):
                """QKV^T for batch b. If fillers is not None, emit closures
                that each do one accumulation-group's worth of work so the
                caller can interleave them into attention; else emit now."""
                ops = []
                for part, dst in ((0, qt[b]), (1, kt[b]), (2, None)):
                    def group(part=part, dst=dst, b=b):
                        ps = pq_pool.tile([128, S], F32, tag="pq", name=f"qkv{b}_{part}")
                        for dt in range(NDT):
                            for nb in range(2):
                                nc.tensor.matmul(
                                    ps[:, nb * 512:(nb + 1) * 512],
                                    wkb[:, dt * 384 + part * 128:
                                        dt * 384 + (part + 1) * 128],
                                    xt[b][:, dt * S + nb * 512:
                                          dt * S + (nb + 1) * 512],
                                    start=(dt == 0),
                                    stop=(dt == NDT - 1),
                                )
                        if dst is not None:
                            nc.vector.tensor_copy(dst[:], ps[:])
                            return None
                        v = vt_pool.tile([128, S], BF, tag="vt", name=f"vt{b}")
                        nc.vector.tensor_copy(v[:], ps[:])
                        return v

                    ops.append(group)

                return ops

            vt_tiles = {}

            def emit_qkv_group(fn, b):
                v = fn()
                if v is not None:
                    vt_tiles[b] = v

            def emit_transpose(b, tb):
                pst = sc_pool.tile([128, 128], BF, tag="sc", name=f"pst{b}_{tb}")
                nc.tensor.transpose(
                    pst[:], vt_tiles[b][:, tb * 128:(tb + 1) * 128], idt[:]
                )
                dst = v65[b][tb]
                for hl in range(2):
                    nc.vector.tensor_copy(
                        dst[:, hl * 65:hl * 65 + 64],
                        pst[:, hl * 64:(hl + 1) * 64],
                    )
                    nc.vector.tensor_copy(
                        dst[:, hl * 65 + 64:hl * 65 + 65],
                        ones_b[:, hl:hl + 1],
                    )

            def scores_phase(b, hl, fillers):
                """scores + exp for all k-blocks of one (batch, head) instance;
                pops one filler closure per k-block to keep PE fed while ACT
                paces the exps."""
                p0 = hl * 64
                ets = []
                for kb in range(NT):
                    psc = sc_pool.tile([128, S], F32, tag="sc",
                                       name=f"psc{b}_{hl}_{kb}")
                    for nb in range(2):
                        nc.tensor.matmul(
                            psc[:, nb * 512:(nb + 1) * 512],
                            kt[b][p0:p0 + 64, kb * 128:(kb + 1) * 128],
                            qt[b][p0:p0 + 64, nb * 512:(nb + 1) * 512],
                            start=True,
                            stop=True,
                        )
                    et = exp_pool.tile([128, S], BF, tag="expt",
                                       name=f"et{b}_{hl}_{kb}")
                    nc.scalar.activation(et[:], psc[:], EXP)
                    ets.append(et)
                    if fillers:
                        fillers.pop(0)()
                return ets

            def attnv_phase(b, hl, ets, fillers):
                """attn@V accumulation + normalization for one instance."""
                pav = pa_pool.tile([65, S], F32, tag="pa", name=f"pav{b}_{hl}")
                for kb in range(NT):
                    for nb in range(2):
                        nc.tensor.matmul(
                            pav[:, nb * 512:(nb + 1) * 512],
                            v65[b][kb][:, hl * 65:(hl + 1) * 65],
                            ets[kb][:, nb * 512:(nb + 1) * 512],
                            start=(kb == 0),
                            stop=(kb == NT - 1),
                        )
                    if fillers:
                        fillers.pop(0)()
                # normalize: f32r reciprocal + PE broadcast + multiply
                rc = rcp_pool.tile([65, S], F32R, tag="rcp", name=f"rc{b}_{hl}")
                nc.vector.reciprocal(rc[64:65, :], pav[64:65, :].bitcast(F32R))
                pbc = sc_pool.tile([64, S], F32, tag="sc", name=f"pbc{b}_{hl}")
                for nb in range(2):
                    nc.tensor.matmul(
                        pbc[:, nb * 512:(nb + 1) * 512],
                        ones_f[64:65, 2:66],
                        rc[64:65, nb * 512:(nb + 1) * 512],
                        start=True,
                        stop=True,
                    )
                bc = bcs_pool.tile([64, S], F32, tag="bcs", name=f"bc{b}_{hl}")
                nc.vector.tensor_copy(bc[:], pbc[:])
                nc.vector.tensor_mul(
                    att[hl][:, b * S:(b + 1) * S], pav[:64, :], bc[:]
                )

            def a2a_combined():
                for hl in range(2):
                    nc.sync.dma_start(
                        a2a_in[it][:].rearrange(
                            "(j r p) s -> r p j s", j=N_CORES, r=2, p=64)[hl],
                        att[hl][:].rearrange("p (j s) -> p j s", s=256),
                    )
                nc.gpsimd.collective_compute(
                    "AllToAll",
                    mybir.AluOpType.bypass,
                    replica_groups=[list(range(N_CORES))],
                    ins=[a2a_in[it][:]],
                    outs=[a2a_out[it][:]],
                )
                af = af_pool.tile([128, N_CORES * 256], BF, tag="af", name="afb")
                nc.sync.dma_start(
                    af[:].rearrange("p (dt s) -> p dt s", s=256),
                    a2a_out[it][:].rearrange("(dt p) s -> p dt s", p=128),
                )
                return af

            def a2a(hl):
                nc.sync.dma_start(
                    a2a_in[it][hl][:].rearrange("(j p) s -> p j s", p=64),
                    att[hl][:].rearrange("p (j s) -> p j s", s=256),
                )
                nc.gpsimd.collective_compute(
                    "AllToAll",
                    mybir.AluOpType.bypass,
                    replica_groups=[list(range(N_CORES))],
                    ins=[a2a_in[it][hl][:]],
                    outs=[a2a_out[it][hl][:]],
                )
                af = af_pool.tile([128, N_CORES * 256], BF, tag="af",
                                  name=f"af{hl}")
                nc.sync.dma_start(
                    af[hl * 64:(hl + 1) * 64, :].rearrange(
                        "p (j s) -> p j s", s=256),
                    a2a_out[it][hl][:].rearrange("(j p) s -> p j s", p=64),
                )
                return af

            # ---------------- emission ----------------
            # batch-0 Q and K eagerly; everything else slots in as PE filler
            # between ACT-paced score/exp blocks.
            g0 = qkv_batch(0)
            g1 = qkv_batch(1)
            emit_qkv_group(g0[0], 0)   # Q(b0)
            emit_qkv_group(g0[1], 0)   # K(b0)

            fillers = [lambda: emit_qkv_group(g0[2], 0)]          # V(b0)
            fillers += [lambda tb=tb: emit_transpose(0, tb) for tb in range(NT)]
            ets00 = scores_phase(0, 0, fillers)        # pops V(b0)+T(b0)x7
            # Q/K of batch 1 right after batch-0 scores: PE fills the stretch
            # where ACT still works through the batch-0 exps.
            emit_qkv_group(g1[0], 1)
            emit_qkv_group(g1[1], 1)
            fillers += [lambda: emit_qkv_group(g1[2], 1)]         # V(b1)
            fillers += [lambda tb=tb: emit_transpose(1, tb) for tb in range(NT)]
            attnv_phase(0, 0, ets00, fillers)
            ets10 = scores_phase(1, 0, fillers)
            attnv_phase(1, 0, ets10, fillers)
            while fillers:
                fillers.pop(0)()

            if SPLIT_A2A:
                af0 = a2a(0)   # hl=0 payload ships while hl=1 computes

            ets01 = scores_phase(0, 1, None)
            attnv_phase(0, 1, ets01, None)
            ets11 = scores_phase(1, 1, None)
            attnv_phase(1, 1, ets11, None)

            if SPLIT_A2A:
                af1 = a2a(1)
                # two K=64 out-projection passes over the A2A payloads
                opart = [osb_pool.tile([128, 256], F32, tag="opart",
                                       name=f"op{eb}", bufs=NDT)
                         for eb in range(NDT)]
                for eb in range(NDT):
                    pool, tag = (pq_pool, "pq") if eb % 2 == 0 else (pa_pool, "pa")
                    po = pool.tile([128, 256], F32, tag=tag, name=f"po1_{eb}")
                    for j in range(N_CORES):
                        nc.tensor.matmul(
                            po[:],
                            wob[0:64, j * D + eb * 128:j * D + (eb + 1) * 128],
                            af0[0:64, j * 256:(j + 1) * 256],
                            start=(j == 0),
                            stop=(j == N_CORES - 1),
                        )
                    nc.vector.tensor_copy(opart[eb][:], po[:])
                for eb in range(NDT):
                    pool, tag = (pq_pool, "pq") if eb % 2 == 0 else (pa_pool, "pa")
                    po = pool.tile([128, 256], F32, tag=tag, name=f"po2_{eb}")
                    for j in range(N_CORES):
                        nc.tensor.matmul(
                            po[:],
                            wob[64:128, j * D + eb * 128:j * D + (eb + 1) * 128],
                            af1[64:128, j * 256:(j + 1) * 256],
                            start=(j == 0),
                            stop=(j == N_CORES - 1),
                        )
                    ot = osb_pool.tile([128, 256], F32, tag="osb", name=f"ot{eb}")
                    nc.vector.tensor_add(ot[:], po[:], opart[eb][:])
                    nc.sync.dma_start(outT[eb * 128:(eb + 1) * 128, :], ot[:])
            else:
                afb = a2a_combined()
                for eb in range(NDT):
                    pool, tag = (pq_pool, "pq") if eb % 2 == 0 else (pa_pool, "pa")
                    po = pool.tile([128, 256], F32, tag=tag, name=f"po_{eb}")
                    for dt in range(NDT):
                        nc.tensor.matmul(
                            po[:],
                            wob[:, dt * D + eb * 128:dt * D + (eb + 1) * 128],
                            afb[:, dt * 256:(dt + 1) * 256],
                            start=(dt == 0),
                            stop=(dt == NDT - 1),
                        )
                    ot = osb_pool.tile([128, 256], F32, tag="osb", name=f"ot{eb}")
                    nc.vector.tensor_copy(ot[:], po[:])
                    nc.sync.dma_start(outT[eb * 128:(eb + 1) * 128, :], ot[:])

    nc.finalize()
    _nc_cache[key] = nc
    return nc


def prep_inputs(x, W_qkv, W_out):
    """Build per-core input maps (numpy only)."""
    x = np.asarray(x, dtype=np.float32)
    W_qkv = np.asarray(W_qkv, dtype=np.float32)
    W_out = np.asarray(W_out, dtype=np.float32)

    xT = np.ascontiguousarray(x.transpose(0, 2, 1)).astype(BF16)   # [B, D, S]
    woT = np.ascontiguousarray(W_out.T).astype(BF16)               # [D, D]
    ident = np.eye(128, dtype=BF16)
    onesB = np.ones((128, 2), dtype=BF16)
    onesF = np.ones((128, 66), dtype=np.float32)

    Wr = W_qkv.reshape(3, H, DH, D)
    in_maps = []
    for c in range(N_CORES):
        hs = slice(2 * c, 2 * c + 2)
        wq = Wr[0, hs].reshape(128, D) * SCALE
        wkk = Wr[1, hs].reshape(128, D)
        wv = Wr[2, hs].reshape(128, D)
        wqkvT = np.ascontiguousarray(np.concatenate([wq, wkk, wv], 0).T).astype(BF16)
        in_maps.append({
            "xT": xT,
            "wqkvT": wqkvT,
            "woT": woT,
            "ident": ident,
            "onesB": onesB,
            "onesF": onesF,
        })
    return in_maps


def assemble(results):
    out = np.empty((B, S, D), dtype=np.float32)
    for c in range(N_CORES):
        b, r = c // 4, c % 4
        out[b, r * 256:(r + 1) * 256, :] = results[c]["outT"].T
    return out


def kernel(x, W_qkv, W_out):
    from concourse.bass_utils import run_bass_kernel_spmd

    nc = _build_nc()
    in_maps = prep_inputs(x, W_qkv, W_out)
    res = run_bass_kernel_spmd(nc, in_maps, list(range(N_CORES)))
    return assemble(res.results)
